# revision 7
# baseline (speedup 1.0000x reference)
"""Trainium2 Bass kernel for the ChebyshevBasis layer.

Reference computation (B=8192, F_IN=F_OUT=1024, D=10, S=1):
    t = tanh(x);  T_0..T_10 Chebyshev polys of t
    coeffs = coeff_mean + eps[0] * exp(0.5*coeff_log_var)      # [F_in, F_out, 11]
    out[b,o] = sum_{i,n} T_n(t[b,i]) * coeffs[i,o,n] + (x @ base_weight)[b,o]
    kl = 0.5 * sum(var + mean^2 - 1 - log_var)

Strategy:
  - Data-parallel over batch: 8 cores x 1024 rows each.
  - Host folds coeffs into a single weight matrix W[(n,i), o] with 11
    contraction slices: T_1..T_10 terms plus the linear (x @ base_weight)
    term.  The T_0 == 1 term reduces to a bias row added on the host.
  - Device: per 128-row F_in phase, compute tanh + Chebyshev recurrence on
    Vector/Act engines in [i, b] layout (x is shipped pre-transposed), then
    run the K=11*1024 contraction as a stream of 128x128x512 matmuls in
    float32r (full-rate fp32 streaming mode, ~1e-4 matmul error) into 8 PSUM
    banks (one per 128-row batch window).
  - KL is a cheap scalar over host-resident tensors; computed on host.
"""
import sys
from contextlib import ExitStack

import numpy as np

if "/opt/trn_rl_repo" not in sys.path:
    sys.path.insert(0, "/opt/trn_rl_repo")

B = 8192
F_IN = 1024
F_OUT = 1024
DEGREE = 10
N_CORES = 8
B_LOC = B // N_CORES          # 1024 batch rows per core
NSL = DEGREE + 1              # 11 contraction slices: T_1..T_10, x
P = 128                       # partition dim
NT = 512                      # matmul free dim / PSUM bank width (fp32)
I_T = F_IN // P               # 8 F_in phases
B_T = B_LOC // P              # 8 batch windows
O_T = F_OUT // NT             # 2 F_out chunks

_CACHE = {}


def _build_program():
    import concourse.tile as tile
    from concourse import mybir, bacc

    dt = mybir.dt
    nc = bacc.Bacc("TRN2", target_bir_lowering=False, debug=False)
    xt_d = nc.dram_tensor("xt", [F_IN, B_LOC], dt.float32r, kind="ExternalInput").ap()
    w_d = nc.dram_tensor(
        "w", [NSL * F_IN, F_OUT], dt.float32r, kind="ExternalInput"
    ).ap()
    out_d = nc.dram_tensor(
        "out", [B_LOC, F_OUT], dt.float32, kind="ExternalOutput"
    ).ap()

    with tile.TileContext(nc) as tc, ExitStack() as ctx:
        wpool = ctx.enter_context(tc.tile_pool(name="wpool", bufs=22))
        bpool = ctx.enter_context(tc.tile_pool(name="bpool", bufs=20))
        spool = ctx.enter_context(tc.tile_pool(name="spool", bufs=2))
        xpool = ctx.enter_context(tc.tile_pool(name="xpool", bufs=2))
        opool = ctx.enter_context(tc.tile_pool(name="opool", bufs=4))
        ppool = ctx.enter_context(tc.tile_pool(name="ppool", bufs=8, space="PSUM"))

        for ot in range(O_T):
            psum = [
                ppool.tile([P, NT], dt.float32, tag="acc", name=f"acc_{ot}_{bt}")
                for bt in range(B_T)
            ]
            for it in range(I_T):
                xw = xpool.tile([P, B_LOC], dt.float32r, tag="xw")
                nc.sync.dma_start(xw[:], xt_d[it * P : (it + 1) * P, :])
                t = spool.tile([P, B_LOC], dt.float32r, tag="t")
                nc.scalar.activation(t[:], xw[:], mybir.ActivationFunctionType.Tanh)
                u = spool.tile([P, B_LOC], dt.float32r, tag="u")
                nc.vector.tensor_scalar_mul(u[:], t[:], 2.0)

                slices = [t]
                tm2, tm1 = None, t
                for _k in range(2, DEGREE + 1):
                    tk = bpool.tile([P, B_LOC], dt.float32r, tag="tk")
                    nc.vector.tensor_mul(tk[:], u[:], tm1[:])
                    if tm2 is None:  # T_2 = 2*t*t - 1
                        nc.vector.tensor_scalar_sub(tk[:], tk[:], 1.0)
                    else:
                        nc.vector.tensor_sub(tk[:], tk[:], tm2[:])
                    slices.append(tk)
                    tm2, tm1 = tm1, tk
                slices.append(xw)  # linear term

                wts = []
                for n in range(NSL):
                    wt = wpool.tile([P, NT], dt.float32r, tag="wt")
                    nc.sync.dma_start(
                        wt[:],
                        w_d[
                            n * F_IN + it * P : n * F_IN + (it + 1) * P,
                            ot * NT : (ot + 1) * NT,
                        ],
                    )
                    wts.append(wt)

                for n in range(NSL):
                    lhs = slices[n][:]
                    for bt in range(B_T):
                        nc.tensor.matmul(
                            psum[bt][:],
                            lhsT=lhs[:, bt * P : (bt + 1) * P],
                            rhs=wts[n][:],
                            start=(it == 0 and n == 0),
                            stop=(it == I_T - 1 and n == NSL - 1),
                        )

            for bt in range(B_T):
                ob = opool.tile([P, NT], dt.float32, tag="ob")
                nc.vector.tensor_copy(ob[:], psum[bt][:])
                nc.sync.dma_start(
                    out_d[bt * P : (bt + 1) * P, ot * NT : (ot + 1) * NT], ob[:]
                )

    nc.compile()
    return nc


def _get_program():
    if "nc" not in _CACHE:
        _CACHE["nc"] = _build_program()
    return _CACHE["nc"]


LAST_EXEC_NS = None
LAST_TRACE = None


def _run(in_maps, trace=False):
    from concourse.bass_utils import run_bass_kernel_spmd

    global LAST_EXEC_NS, LAST_TRACE
    nc = _get_program()
    res = run_bass_kernel_spmd(
        nc, in_maps, core_ids=list(range(N_CORES)), trace=trace
    )
    LAST_EXEC_NS = res.exec_time_ns
    if res.instructions_and_trace is not None:
        LAST_TRACE = res.instructions_and_trace[1]
    return res


TRACE = False


def kernel(x, coeff_mean, coeff_log_var, base_weight, eps):
    x = np.asarray(x, dtype=np.float32)
    coeff_mean = np.asarray(coeff_mean, dtype=np.float32)
    coeff_log_var = np.asarray(coeff_log_var, dtype=np.float32)
    base_weight = np.asarray(base_weight, dtype=np.float32)
    eps = np.asarray(eps, dtype=np.float32)

    # Host-side coefficient fusion (elementwise, ~0.05% of total FLOPs).
    std = np.exp(0.5 * coeff_log_var)
    coeffs = coeff_mean + eps[0] * std                  # [F_in, F_out, 11]
    ct = np.ascontiguousarray(coeffs.transpose(2, 0, 1))  # [11, F_in, F_out]
    w = np.empty((NSL * F_IN, F_OUT), dtype=np.float32)
    w[: DEGREE * F_IN] = ct[1:].reshape(DEGREE * F_IN, F_OUT)  # T_1..T_10
    w[DEGREE * F_IN :] = base_weight                           # linear term
    bias = ct[0].sum(axis=0)                                   # T_0 == 1 term

    var = np.exp(coeff_log_var)
    kl = np.float32(
        0.5
        * np.sum(
            (var + coeff_mean.astype(np.float64) ** 2 - 1.0 - coeff_log_var),
            dtype=np.float64,
        )
    )

    in_maps = []
    for c in range(N_CORES):
        xt_c = np.ascontiguousarray(x[c * B_LOC : (c + 1) * B_LOC].T)
        in_maps.append({"xt": xt_c, "w": w})

    res = _run(in_maps, trace=TRACE)

    out = np.empty((B, F_OUT), dtype=np.float32)
    for c in range(N_CORES):
        out[c * B_LOC : (c + 1) * B_LOC] = res.results[c]["out"]
    out += bias[None, :]
    return out, kl


# revision 8
# speedup vs baseline: 1.2402x; 1.2402x over previous
"""Trainium2 Bass kernel for the ChebyshevBasis layer.

Reference computation (B=8192, F_IN=F_OUT=1024, D=10, S=1):
    t = tanh(x);  T_0..T_10 Chebyshev polys of t
    coeffs = coeff_mean + eps[0] * exp(0.5*coeff_log_var)      # [F_in, F_out, 11]
    out[b,o] = sum_{i,n} T_n(t[b,i]) * coeffs[i,o,n] + (x @ base_weight)[b,o]
    kl = 0.5 * sum(var + mean^2 - 1 - log_var)

Strategy:
  - Data-parallel over batch: 8 cores x 1024 rows each.
  - Host folds coeffs into a single weight matrix W[(n,i), o] with 11
    contraction slices: T_1..T_10 terms plus the linear (x @ base_weight)
    term.  The T_0 == 1 term reduces to a bias row added on the host.
  - Device: per 128-row F_in phase, compute tanh + Chebyshev recurrence on
    Vector/Act engines in [i, b] layout (x is shipped pre-transposed), then
    run the K=11*1024 contraction as a stream of 128x128x512 matmuls in
    float16 (full-rate streaming, 10-bit mantissa) into 8 PSUM
    banks (one per 128-row batch window).
  - KL is a cheap scalar over host-resident tensors; computed on host.
"""
import sys
from contextlib import ExitStack

import numpy as np

if "/opt/trn_rl_repo" not in sys.path:
    sys.path.insert(0, "/opt/trn_rl_repo")

B = 8192
F_IN = 1024
F_OUT = 1024
DEGREE = 10
N_CORES = 8
B_LOC = B // N_CORES          # 1024 batch rows per core
NSL = DEGREE + 1              # 11 contraction slices: T_1..T_10, x
P = 128                       # partition dim
NT = 512                      # matmul free dim / PSUM bank width (fp32)
I_T = F_IN // P               # 8 F_in phases
B_T = B_LOC // P              # 8 batch windows
O_T = F_OUT // NT             # 2 F_out chunks

_CACHE = {}


def _build_program():
    import concourse.tile as tile
    from concourse import mybir, bacc

    dt = mybir.dt
    nc = bacc.Bacc("TRN2", target_bir_lowering=False, debug=False)
    xt_d = nc.dram_tensor("xt", [F_IN, B_LOC], dt.float16, kind="ExternalInput").ap()
    w_d = nc.dram_tensor(
        "w", [NSL * F_IN, F_OUT], dt.float16, kind="ExternalInput"
    ).ap()
    out_d = nc.dram_tensor(
        "out", [B_LOC, F_OUT], dt.float32, kind="ExternalOutput"
    ).ap()

    with tile.TileContext(nc) as tc, ExitStack() as ctx:
        wpool = ctx.enter_context(tc.tile_pool(name="wpool", bufs=33))
        bpool = ctx.enter_context(tc.tile_pool(name="bpool", bufs=20))
        spool = ctx.enter_context(tc.tile_pool(name="spool", bufs=2))
        xpool = ctx.enter_context(tc.tile_pool(name="xpool", bufs=2))
        opool = ctx.enter_context(tc.tile_pool(name="opool", bufs=4))
        ppool = ctx.enter_context(tc.tile_pool(name="ppool", bufs=8, space="PSUM"))

        for ot in range(O_T):
            psum = [
                ppool.tile([P, NT], dt.float32, tag="acc", name=f"acc_{ot}_{bt}")
                for bt in range(B_T)
            ]
            for it in range(I_T):
                xw = xpool.tile([P, B_LOC], dt.float16, tag="xw")
                nc.sync.dma_start(xw[:], xt_d[it * P : (it + 1) * P, :])
                t = spool.tile([P, B_LOC], dt.float16, tag="t")
                nc.scalar.activation(t[:], xw[:], mybir.ActivationFunctionType.Tanh)
                u = spool.tile([P, B_LOC], dt.float16, tag="u")
                nc.vector.tensor_scalar_mul(u[:], t[:], 2.0)

                slices = [t]
                tm2, tm1 = None, t
                for _k in range(2, DEGREE + 1):
                    tk = bpool.tile([P, B_LOC], dt.float16, tag="tk")
                    nc.vector.tensor_mul(tk[:], u[:], tm1[:])
                    if tm2 is None:  # T_2 = 2*t*t - 1
                        nc.vector.tensor_scalar_sub(tk[:], tk[:], 1.0)
                    else:
                        nc.vector.tensor_sub(tk[:], tk[:], tm2[:])
                    slices.append(tk)
                    tm2, tm1 = tm1, tk
                slices.append(xw)  # linear term

                wts = []
                for n in range(NSL):
                    wt = wpool.tile([P, NT], dt.float16, tag="wt")
                    nc.sync.dma_start(
                        wt[:],
                        w_d[
                            n * F_IN + it * P : n * F_IN + (it + 1) * P,
                            ot * NT : (ot + 1) * NT,
                        ],
                    )
                    wts.append(wt)

                for n in range(NSL):
                    lhs = slices[n][:]
                    for bt in range(B_T):
                        nc.tensor.matmul(
                            psum[bt][:],
                            lhsT=lhs[:, bt * P : (bt + 1) * P],
                            rhs=wts[n][:],
                            start=(it == 0 and n == 0),
                            stop=(it == I_T - 1 and n == NSL - 1),
                        )

            for bt in range(B_T):
                ob = opool.tile([P, NT], dt.float32, tag="ob")
                nc.vector.tensor_copy(ob[:], psum[bt][:])
                nc.sync.dma_start(
                    out_d[bt * P : (bt + 1) * P, ot * NT : (ot + 1) * NT], ob[:]
                )

    nc.compile()
    return nc


def _get_program():
    if "nc" not in _CACHE:
        _CACHE["nc"] = _build_program()
    return _CACHE["nc"]


LAST_EXEC_NS = None
LAST_TRACE = None


def _run(in_maps, trace=False):
    from concourse.bass_utils import run_bass_kernel_spmd

    global LAST_EXEC_NS, LAST_TRACE
    nc = _get_program()
    res = run_bass_kernel_spmd(
        nc, in_maps, core_ids=list(range(N_CORES)), trace=trace
    )
    LAST_EXEC_NS = res.exec_time_ns
    if res.instructions_and_trace is not None:
        LAST_TRACE = res.instructions_and_trace[1]
    return res


TRACE = False


def kernel(x, coeff_mean, coeff_log_var, base_weight, eps):
    x = np.asarray(x, dtype=np.float32)
    coeff_mean = np.asarray(coeff_mean, dtype=np.float32)
    coeff_log_var = np.asarray(coeff_log_var, dtype=np.float32)
    base_weight = np.asarray(base_weight, dtype=np.float32)
    eps = np.asarray(eps, dtype=np.float32)

    # Host-side coefficient fusion (elementwise, ~0.05% of total FLOPs).
    std = np.exp(0.5 * coeff_log_var)
    coeffs = coeff_mean + eps[0] * std                  # [F_in, F_out, 11]
    ct = np.ascontiguousarray(coeffs.transpose(2, 0, 1))  # [11, F_in, F_out]
    w = np.empty((NSL * F_IN, F_OUT), dtype=np.float32)
    w[: DEGREE * F_IN] = ct[1:].reshape(DEGREE * F_IN, F_OUT)  # T_1..T_10
    w[DEGREE * F_IN :] = base_weight                           # linear term
    bias = ct[0].sum(axis=0)                                   # T_0 == 1 term
    w16 = w.astype(np.float16)

    var = np.exp(coeff_log_var)
    kl = np.float32(
        0.5
        * np.sum(
            (var + coeff_mean.astype(np.float64) ** 2 - 1.0 - coeff_log_var),
            dtype=np.float64,
        )
    )

    in_maps = []
    for c in range(N_CORES):
        xt_c = np.ascontiguousarray(x[c * B_LOC : (c + 1) * B_LOC].T).astype(np.float16)
        in_maps.append({"xt": xt_c, "w": w16})

    res = _run(in_maps, trace=TRACE)

    out = np.empty((B, F_OUT), dtype=np.float32)
    for c in range(N_CORES):
        out[c * B_LOC : (c + 1) * B_LOC] = res.results[c]["out"]
    out += bias[None, :]
    return out, kl


# revision 9
# speedup vs baseline: 1.2676x; 1.0221x over previous
"""Trainium2 Bass kernel for the ChebyshevBasis layer.

Reference computation (B=8192, F_IN=F_OUT=1024, D=10, S=1):
    t = tanh(x);  T_0..T_10 Chebyshev polys of t
    coeffs = coeff_mean + eps[0] * exp(0.5*coeff_log_var)      # [F_in, F_out, 11]
    out[b,o] = sum_{i,n} T_n(t[b,i]) * coeffs[i,o,n] + (x @ base_weight)[b,o]
    kl = 0.5 * sum(var + mean^2 - 1 - log_var)

Strategy:
  - Data-parallel over batch: 8 cores x 1024 rows each.
  - Host folds coeffs into a single weight matrix W[(n,i), o] with 11
    contraction slices: T_1..T_10 terms plus the linear (x @ base_weight)
    term.  The T_0 == 1 term reduces to a bias row added on the host.
  - Device: per 128-row F_in phase, compute tanh + Chebyshev recurrence on
    Vector/Act engines in [i, b] layout (x is shipped pre-transposed), then
    run the K=11*1024 contraction as a stream of 128x128x512 matmuls in
    float16 (full-rate streaming, 10-bit mantissa) into 8 PSUM
    banks (one per 128-row batch window).
  - KL is a cheap scalar over host-resident tensors; computed on host.
"""
import sys
from contextlib import ExitStack

import numpy as np

if "/opt/trn_rl_repo" not in sys.path:
    sys.path.insert(0, "/opt/trn_rl_repo")

B = 8192
F_IN = 1024
F_OUT = 1024
DEGREE = 10
N_CORES = 8
B_LOC = B // N_CORES          # 1024 batch rows per core
NSL = DEGREE + 1              # 11 contraction slices: T_1..T_10, x
P = 128                       # partition dim
NT = 512                      # matmul free dim / PSUM bank width (fp32)
I_T = F_IN // P               # 8 F_in phases
B_T = B_LOC // P              # 8 batch windows
O_T = F_OUT // NT             # 2 F_out chunks

_CACHE = {}


def _build_program():
    import concourse.tile as tile
    from concourse import mybir, bacc

    dt = mybir.dt
    nc = bacc.Bacc("TRN2", target_bir_lowering=False, debug=False)
    xt_d = nc.dram_tensor("xt", [F_IN, B_LOC], dt.float16, kind="ExternalInput").ap()
    w_d = nc.dram_tensor(
        "w", [NSL * F_IN, F_OUT], dt.float16, kind="ExternalInput"
    ).ap()
    out_d = nc.dram_tensor(
        "out", [B_LOC, F_OUT], dt.float32, kind="ExternalOutput"
    ).ap()

    with tile.TileContext(nc) as tc, ExitStack() as ctx:
        wpool = ctx.enter_context(tc.tile_pool(name="wpool", bufs=33))
        bpool = ctx.enter_context(tc.tile_pool(name="bpool", bufs=20))
        spool = ctx.enter_context(tc.tile_pool(name="spool", bufs=2))
        xpool = ctx.enter_context(tc.tile_pool(name="xpool", bufs=2))
        opool = ctx.enter_context(tc.tile_pool(name="opool", bufs=4))
        ppool = ctx.enter_context(tc.tile_pool(name="ppool", bufs=8, space="PSUM"))

        for ot in range(O_T):
            psum = [
                ppool.tile([P, NT], dt.float32, tag="acc", name=f"acc_{ot}_{bt}")
                for bt in range(B_T)
            ]
            for it in range(I_T):
                xw = xpool.tile([P, B_LOC], dt.float16, tag="xw")
                nc.sync.dma_start(xw[:], xt_d[it * P : (it + 1) * P, :])
                t = spool.tile([P, B_LOC], dt.float16, tag="t")
                nc.scalar.activation(t[:], xw[:], mybir.ActivationFunctionType.Tanh)
                u = spool.tile([P, B_LOC], dt.float16, tag="u")
                nc.vector.tensor_scalar_mul(u[:], t[:], 2.0)

                slices = [t]
                tm2, tm1 = None, t
                for _k in range(2, DEGREE + 1):
                    tk = bpool.tile([P, B_LOC], dt.float16, tag="tk")
                    nc.vector.tensor_mul(tk[:], u[:], tm1[:])
                    if tm2 is None:  # T_2 = 2*t*t - 1
                        nc.vector.tensor_scalar_sub(tk[:], tk[:], 1.0)
                    else:
                        nc.vector.tensor_sub(tk[:], tk[:], tm2[:])
                    slices.append(tk)
                    tm2, tm1 = tm1, tk
                slices.append(xw)  # linear term

                wts = []
                for n in range(NSL):
                    wt = wpool.tile([P, NT], dt.float16, tag="wt")
                    nc.sync.dma_start(
                        wt[:],
                        w_d[
                            n * F_IN + it * P : n * F_IN + (it + 1) * P,
                            ot * NT : (ot + 1) * NT,
                        ],
                    )
                    wts.append(wt)

                if it < I_T - 1:
                    mm_order = [(n, bt) for n in range(NSL) for bt in range(B_T)]
                else:
                    # last phase: bank-major so early banks finish first and
                    # their PSUM evicts overlap the remaining matmuls
                    mm_order = [(n, bt) for bt in range(B_T) for n in range(NSL)]
                for n, bt in mm_order:
                    nc.tensor.matmul(
                        psum[bt][:],
                        lhsT=slices[n][:][:, bt * P : (bt + 1) * P],
                        rhs=wts[n][:],
                        start=(it == 0 and n == 0),
                        stop=(it == I_T - 1 and n == NSL - 1),
                    )

            for bt in range(B_T):
                ob = opool.tile([P, NT], dt.float32, tag="ob")
                nc.vector.tensor_copy(ob[:], psum[bt][:])
                nc.sync.dma_start(
                    out_d[bt * P : (bt + 1) * P, ot * NT : (ot + 1) * NT], ob[:]
                )

    nc.compile()
    return nc


def _get_program():
    if "nc" not in _CACHE:
        _CACHE["nc"] = _build_program()
    return _CACHE["nc"]


LAST_EXEC_NS = None
LAST_TRACE = None


def _run(in_maps, trace=False):
    from concourse.bass_utils import run_bass_kernel_spmd

    global LAST_EXEC_NS, LAST_TRACE
    nc = _get_program()
    res = run_bass_kernel_spmd(
        nc, in_maps, core_ids=list(range(N_CORES)), trace=trace
    )
    LAST_EXEC_NS = res.exec_time_ns
    if res.instructions_and_trace is not None:
        LAST_TRACE = res.instructions_and_trace[1]
    return res


TRACE = False


def kernel(x, coeff_mean, coeff_log_var, base_weight, eps):
    x = np.asarray(x, dtype=np.float32)
    coeff_mean = np.asarray(coeff_mean, dtype=np.float32)
    coeff_log_var = np.asarray(coeff_log_var, dtype=np.float32)
    base_weight = np.asarray(base_weight, dtype=np.float32)
    eps = np.asarray(eps, dtype=np.float32)

    # Host-side coefficient fusion (elementwise, ~0.05% of total FLOPs).
    std = np.exp(0.5 * coeff_log_var)
    coeffs = coeff_mean + eps[0] * std                  # [F_in, F_out, 11]
    ct = np.ascontiguousarray(coeffs.transpose(2, 0, 1))  # [11, F_in, F_out]
    w = np.empty((NSL * F_IN, F_OUT), dtype=np.float32)
    w[: DEGREE * F_IN] = ct[1:].reshape(DEGREE * F_IN, F_OUT)  # T_1..T_10
    w[DEGREE * F_IN :] = base_weight                           # linear term
    bias = ct[0].sum(axis=0)                                   # T_0 == 1 term
    w16 = w.astype(np.float16)

    var = np.exp(coeff_log_var)
    kl = np.float32(
        0.5
        * np.sum(
            (var + coeff_mean.astype(np.float64) ** 2 - 1.0 - coeff_log_var),
            dtype=np.float64,
        )
    )

    in_maps = []
    for c in range(N_CORES):
        xt_c = np.ascontiguousarray(x[c * B_LOC : (c + 1) * B_LOC].T).astype(np.float16)
        in_maps.append({"xt": xt_c, "w": w16})

    res = _run(in_maps, trace=TRACE)

    out = np.empty((B, F_OUT), dtype=np.float32)
    for c in range(N_CORES):
        out[c * B_LOC : (c + 1) * B_LOC] = res.results[c]["out"]
    out += bias[None, :]
    return out, kl
